# revision 3
# baseline (speedup 1.0000x reference)
"""Trainium2 kernel for nn_DoubleAffineNet — v5.

Same math as v4 (fp16 stream, single sync HWDGE ring, DVE+ACT+PE
three-engine reduce, host finishes the O(KB) algebra). Tail changes
driven by the v4 traces:

  - X3b tail chunk back on PE[0:256]+DVE[256:512] (the v4.1 ACT-serial
    tail gated C2 ~1us late)
  - the PSUM accumulators are no longer reduced on-device at the tail:
    DVE just COPIES psY/psX [1,512] rows into an SBUF staging row and
    the 1024 raw partials ship as a single-descriptor DMA on the
    otherwise-idle ACT ring; the host sums them (same class of host
    work as the border strips). This removes the 0.8us serial PSUM
    reduce from the critical path and shrinks C2 to [128,3].
"""

import numpy as np

H = 1024
W = 1024
OUT_F32 = 2304   # [128,7] C1 + [128,3] C2 + [1,1024] raw psum partials
OUT_F16 = 4096   # [128,16] col strips + 1024 row0 + 1024 row1023

FINAL_WAIT = False

_CACHE = {}


def _build_program(final_wait=False):
    import contextlib

    import concourse.bacc as bacc
    from concourse import mybir

    f16 = mybir.dt.float16
    f32 = mybir.dt.float32
    Copy = mybir.ActivationFunctionType.Copy
    nc = bacc.Bacc(
        "TRN2",
        target_bir_lowering=False,
        debug=False,
        num_devices=8,
        enable_partition_id=False,
    )

    xd = nc.dram_tensor("x", [H, W], f16, kind="ExternalInput").ap()
    yd = nc.dram_tensor("y", [H, W], f16, kind="ExternalInput").ap()
    outd = nc.dram_tensor("out", [OUT_F32], f32, kind="ExternalOutput").ap()
    outs = nc.dram_tensor("outs", [OUT_F16], f16, kind="ExternalOutput").ap()

    CH = [
        ("Y0", "y", 0, 512),
        ("Y1", "y", 512, 512),
        ("X0", "x", 0, 512),
        ("X1", "x", 512, 256),
        ("X2", "x", 768, 128),
        ("X3a", "x", 896, 64),
        ("X3b", "x", 960, 64),
    ]
    names = [c[0] for c in CH]
    wid = {n: nr * W // 128 for (n, _, _, nr) in CH}

    def src_ap(tensor, r0, nrows):
        td = xd if tensor == "x" else yd
        sl = td[r0 : r0 + nrows, :]
        if nrows > 128:
            return sl.rearrange("(p a) q -> p (a q)", a=nrows // 128)
        if nrows == 128:
            return sl
        return sl.rearrange("r (h q) -> (r h) q", h=2)

    # smalls [128,10] f32:
    #  C1 cols 0..6: Y0d, Y1d, X0d, X1d, Y0b, Y1b, X0b
    #  C2 cols 7..9: X2b, X3ad, X3bd
    with contextlib.ExitStack() as ctx:
        bufs = {
            n: ctx.enter_context(nc.sbuf_tensor(f"b_{n}", [128, wid[n]], f16))
            for n in names
        }
        smalls = ctx.enter_context(nc.sbuf_tensor("smalls", [128, 10], f32))
        strips = ctx.enter_context(nc.sbuf_tensor("strips", [128, 16], f16))
        psstage = ctx.enter_context(nc.sbuf_tensor("psstage", [1, 1024], f32))
        scratch = ctx.enter_context(nc.sbuf_tensor("scratch", [128, 2 * W], f16))
        ones = ctx.enter_context(nc.sbuf_tensor("ones", [128, 1], f16))
        psY = ctx.enter_context(nc.psum_tensor("psY", [128, 512], f32))
        psX = ctx.enter_context(nc.psum_tensor("psX", [128, 512], f32))
        in_sem = {n: ctx.enter_context(nc.semaphore(f"s_{n}")) for n in names}
        done1 = ctx.enter_context(nc.semaphore("done1"))
        done2 = ctx.enter_context(nc.semaphore("done2"))
        done_s = ctx.enter_context(nc.semaphore("done_s"))
        sem_ones = ctx.enter_context(nc.semaphore("sem_ones"))
        pe_y = ctx.enter_context(nc.semaphore("pe_y"))
        pe_x = ctx.enter_context(nc.semaphore("pe_x"))
        ps_done = ctx.enter_context(nc.semaphore("ps_done"))
        dma_out = ctx.enter_context(nc.semaphore("dma_out"))
        block = ctx.enter_context(nc.Block(no_gpsimd_drain=True))

        @block.sync
        def _(sync):
            for (n, t, r0, nr) in CH:
                sync.dma_start(out=bufs[n][:], in_=src_ap(t, r0, nr)).then_inc(
                    in_sem[n], 16
                )
            sync.wait_ge(done1, 7)
            sync.dma_start(
                out=outd[0:896].rearrange("(p c) -> p c", c=7),
                in_=smalls[:, 0:7],
            ).then_inc(dma_out, 16)
            sync.wait_ge(done2, 3)
            sync.dma_start(
                out=outd[896:1280].rearrange("(p c) -> p c", c=3),
                in_=smalls[:, 7:10],
            ).then_inc(dma_out, 16)
            if final_wait:
                sync.wait_ge(dma_out, 96)

        @block.tensor
        def _(tensor):
            def mm(ps, buf, lo, width_, start, stop):
                nc.tensor.matmul(
                    out=ps[0:1, 0:width_],
                    lhsT=ones.ap(),
                    rhs=buf[:, lo : lo + width_],
                    start=start,
                    stop=stop,
                )

            tensor.wait_ge(sem_ones, 1)
            tensor.wait_ge(in_sem["Y0"], 16)
            mm(psY, bufs["Y0"], 0, 512, True, False)
            mm(psY, bufs["Y0"], 512, 512, False, False)
            tensor.wait_ge(in_sem["Y1"], 16)
            mm(psY, bufs["Y1"], 0, 512, False, False)
            nc.tensor.matmul(
                out=psY[0:1, 0:512], lhsT=ones.ap(),
                rhs=bufs["Y1"][:, 512:1024], start=False, stop=True,
            ).then_inc(pe_y, 1)
            tensor.wait_ge(in_sem["X0"], 16)
            mm(psX, bufs["X0"], 0, 512, True, False)
            mm(psX, bufs["X0"], 512, 512, False, False)
            tensor.wait_ge(in_sem["X1"], 16)
            mm(psX, bufs["X1"], 0, 512, False, False)
            mm(psX, bufs["X1"], 512, 512, False, False)
            tensor.wait_ge(in_sem["X2"], 16)
            mm(psX, bufs["X2"], 0, 512, False, False)
            tensor.wait_ge(in_sem["X3a"], 16)
            mm(psX, bufs["X3a"], 0, 256, False, False)
            tensor.wait_ge(in_sem["X3b"], 16)
            nc.tensor.matmul(
                out=psX[0:1, 0:256], lhsT=ones.ap(),
                rhs=bufs["X3b"][:, 0:256], start=False, stop=True,
            ).then_inc(pe_x, 1)

        @block.vector
        def _(vector):
            def red(src_ap_, col, done_sem):
                nc.vector.tensor_reduce(
                    out=smalls[:, col : col + 1],
                    in_=src_ap_,
                    axis=mybir.AxisListType.X,
                    op=mybir.AluOpType.add,
                ).then_inc(done_sem, 1)

            vector.wait_ge(in_sem["Y0"], 16)
            red(bufs["Y0"][:, 1024:2048], 0, done1)
            vector.wait_ge(in_sem["Y1"], 16)
            red(bufs["Y1"][:, 1024:2048], 1, done1)
            vector.wait_ge(pe_y, 1)
            nc.vector.tensor_copy(
                psstage[0:1, 0:512], psY[0:1, 0:512]
            ).then_inc(ps_done, 1)
            vector.wait_ge(in_sem["X0"], 16)
            red(bufs["X0"][:, 1024:2048], 2, done1)
            vector.wait_ge(in_sem["X1"], 16)
            red(bufs["X1"][:, 1024:2048], 3, done1)
            vector.wait_ge(in_sem["X3a"], 16)
            red(bufs["X3a"][:, 256:512], 8, done2)
            vector.wait_ge(in_sem["X3b"], 16)
            red(bufs["X3b"][:, 256:512], 9, done2)
            vector.wait_ge(pe_x, 1)
            nc.vector.tensor_copy(
                psstage[0:1, 512:1024], psX[0:1, 0:512]
            ).then_inc(ps_done, 1)

        @block.scalar
        def _(scalar):
            def act(n, lo, hi, col, done_sem):
                nc.scalar.activation(
                    scratch[:, 0 : hi - lo], bufs[n][:, lo:hi], Copy,
                    accum_out=smalls[:, col : col + 1],
                ).then_inc(done_sem, 1)

            scalar.wait_ge(in_sem["Y0"], 16)
            scalar.dma_start(
                out=outs[2048:3072].rearrange("(p q) -> p q", p=1),
                in_=bufs["Y0"][0:1, 0:W],
            ).then_inc(dma_out, 16)
            act("Y0", 2048, 4096, 4, done1)
            scalar.wait_ge(in_sem["Y1"], 16)
            scalar.dma_start(
                out=outs[3072:4096].rearrange("(p q) -> p q", p=1),
                in_=bufs["Y1"][127:128, 3 * W : 4 * W],
            ).then_inc(dma_out, 16)
            act("Y1", 2048, 4096, 5, done1)
            scalar.wait_ge(done_s, 4)
            scalar.dma_start(
                out=outs[0:2048].rearrange("(p c) -> p c", c=16),
                in_=strips[:],
            ).then_inc(dma_out, 16)
            scalar.wait_ge(in_sem["X0"], 16)
            act("X0", 2048, 4096, 6, done1)
            scalar.wait_ge(in_sem["X2"], 16)
            act("X2", 512, 1024, 7, done2)
            scalar.wait_ge(ps_done, 2)
            scalar.dma_start(
                out=outd[1280:2304].rearrange("(p q) -> p q", p=1),
                in_=psstage[0:1, :],
            ).then_inc(dma_out, 16)

        @block.gpsimd
        def _(gpsimd):
            nc.gpsimd.memset(ones.ap(), 1.0).then_inc(sem_ones, 1)
            for c, n in enumerate(("Y0", "Y1")):
                gpsimd.wait_ge(in_sem[n], 16)
                t4 = bufs[n].ap().rearrange("p (a q) -> p a q", a=4)
                nc.gpsimd.tensor_copy(
                    strips[:, 4 * c : 4 * c + 4], t4[:, :, 0]
                ).then_inc(done_s, 1)
                nc.gpsimd.tensor_copy(
                    strips[:, 8 + 4 * c : 12 + 4 * c], t4[:, :, W - 1]
                ).then_inc(done_s, 1)

    nc.compile()
    return nc


def _get_program():
    key = ("nc", FINAL_WAIT)
    if key not in _CACHE:
        _CACHE[key] = _build_program(final_wait=FINAL_WAIT)
    return _CACHE[key]


def _tent(z):
    return np.maximum(0.0, 1.0 - np.abs(z))


def _warp_mean_exact(y_img, A):
    A64 = A.astype(np.float64)
    i = np.arange(H, dtype=np.float64)[:, None]
    j = np.arange(W, dtype=np.float64)[None, :]
    px = A64[0, 0] * i + A64[0, 1] * j + 1023.0 * A64[0, 2]
    py = A64[1, 0] * i + A64[1, 1] * j + 1023.0 * A64[1, 2]
    x0 = np.floor(px).astype(np.int64)
    y0 = np.floor(py).astype(np.int64)
    wx = px - x0
    wy = py - y0
    im = y_img.astype(np.float64)
    acc = np.zeros((H, W))
    for xi, yi, w in (
        (x0, y0, (1 - wx) * (1 - wy)),
        (x0, y0 + 1, (1 - wx) * wy),
        (x0 + 1, y0, wx * (1 - wy)),
        (x0 + 1, y0 + 1, wx * wy),
    ):
        valid = (xi >= 0) & (xi < H) & (yi >= 0) & (yi < W)
        acc += im[np.clip(xi, 0, H - 1), np.clip(yi, 0, W - 1)] * w * valid
    return acc.mean()


def _warp_sum(sum_y, row0, row1, c0, c1, A):
    A64 = A.astype(np.float64)
    ap, bb = A64[0, 0] - 1.0, A64[0, 1]
    cc, dp = A64[1, 0], A64[1, 1] - 1.0
    e1, e2 = 1023.0 * A64[0, 2], 1023.0 * A64[1, 2]

    mu = max(abs(ap * i + bb * j + e1) for i in (0.0, 1023.0) for j in (0.0, 1023.0))
    mv = max(abs(cc * i + dp * j + e2) for i in (0.0, 1023.0) for j in (0.0, 1023.0))
    assert mu < 0.5 and mv < 0.5, (mu, mv)

    kappa = (1.0 - ap) * (1.0 - dp) + bb * cc

    def g_true(p, q):
        g = np.zeros(np.broadcast(p, q).shape)
        for di in (-1, 0, 1):
            for dj in (-1, 0, 1):
                i_, j_ = p - di, q - dj
                valid = (i_ >= 0) & (i_ < H) & (j_ >= 0) & (j_ < W)
                z1 = ap * i_ + bb * j_ + e1 - di
                z2 = cc * i_ + dp * j_ + e2 - dj
                g += _tent(z1) * _tent(z2) * valid
        return g

    qs = np.arange(W, dtype=np.float64)
    ps = np.arange(1, H - 1, dtype=np.float64)
    ds = 0.0
    ds += np.sum(row0.astype(np.float64) * (g_true(0.0, qs) - kappa))
    ds += np.sum(row1.astype(np.float64) * (g_true(1023.0, qs) - kappa))
    ds += np.sum(c0[1:-1].astype(np.float64) * (g_true(ps, 0.0) - kappa))
    ds += np.sum(c1[1:-1].astype(np.float64) * (g_true(ps, 1023.0) - kappa))

    return kappa * float(sum_y) + ds


def _affine_f32(feat32, Wl, bl):
    M = (feat32 @ Wl + bl).reshape(3, 3)
    return np.eye(3, dtype=np.float32) + np.float32(0.01) * M


def _decode(r32, r16):
    sm1 = r32[0:896].reshape(128, 7).astype(np.float64)
    sm2 = r32[896:1280].reshape(128, 3).astype(np.float64)
    psvec = r32[1280:2304].astype(np.float64)
    sum_y = float(sm1[:, 0:2].sum() + sm1[:, 4:6].sum() + psvec[0:512].sum())
    sum_x = float(
        sm1[:, 2:4].sum() + sm1[:, 6].sum() + sm2.sum() + psvec[512:1024].sum()
    )
    st = r16[0:2048].reshape(128, 16).astype(np.float64)
    c0 = np.concatenate([st[:, 4 * c : 4 * c + 4].ravel() for c in range(2)])
    c1 = np.concatenate([st[:, 8 + 4 * c : 12 + 4 * c].ravel() for c in range(2)])
    row0 = r16[2048:3072].astype(np.float64)
    row1 = r16[3072:4096].astype(np.float64)
    return sum_x, sum_y, row0, row1, c0, c1


def kernel(x, y, Wpsi, bpsi, Wphi, bphi):
    from concourse import bass_utils

    B = x.shape[0]
    assert x.shape == (B, 1, H, W) and y.shape == (B, 1, H, W)

    x16 = x.astype(np.float16)
    y16 = y.astype(np.float16)

    nc = _get_program()
    in_maps = [
        {"x": np.ascontiguousarray(x16[b, 0]), "y": np.ascontiguousarray(y16[b, 0])}
        for b in range(B)
    ]
    results = bass_utils.run_bass_kernel_spmd(
        nc, in_maps, core_ids=list(range(B))
    ).results

    out = np.empty((B, 3, 3), dtype=np.float32)
    inv_hw = 1.0 / float(H * W)
    for b in range(B):
        r32 = np.asarray(results[b]["out"], dtype=np.float32).reshape(-1)
        r16 = np.asarray(results[b]["outs"]).reshape(-1)
        sum_x, sum_y, row0, row1, c0, c1 = _decode(r32, r16)

        mean_x = np.float32(sum_x * inv_hw)
        mean_y = np.float32(sum_y * inv_hw)
        phi = _affine_f32(np.array([mean_x, mean_y], np.float32), Wpsi, bpsi)
        A = np.linalg.inv(phi)

        try:
            mean_yc = np.float32(_warp_sum(sum_y, row0, row1, c0, c1, A) * inv_hw)
        except AssertionError:
            mean_yc = np.float32(_warp_mean_exact(y16[b, 0], A))

        psi = _affine_f32(np.array([mean_x, mean_yc], np.float32), Wphi, bphi)
        out[b] = phi + psi - np.eye(3, dtype=np.float32)
    return out
